# revision 41
# baseline (speedup 1.0000x reference)
"""Trainium2 Bass kernel: BigramHashEmbedding (hash -> embed gather -> proj -> scale).

Computation (per batch row, one NeuronCore per row, 8 rows total):
    h[0]  = 10239
    h[j]  = (36313*t[j] ^ 27191*t[j-1]) % 10239          (int32, j >= 1)
    e     = embed_weight[h]                               [S, 128] gather
    out   = (e @ proj_weight.T) * scale                   [S, 512]

Device strategy per core (S = 8192 tokens):
  * dma_gather unwraps its index tile column-major over 16 partitions
    (slot k <- idx[k%16, k//16]), so the host stages tokens in a 16-wrap
    layout (tok16[p, s] = t[16s + p], plus a one-shifted copy for the
    bigram's previous token; both are pure permutations of the int32 index
    tensor, staged as one [2, 16, 512] input).  With this layout gather
    slot k maps to token k exactly: gathered rows land as
    g_sb[p, b, :] = e[token 128b + p], the PE transpose of block b yields
    eT in plain token order, and every output DMA writes 128 contiguous
    rows (fully sequential HBM addresses).
  * the device loads the two wrapped tiles with contiguous 2KB-per-
    partition runs and broadcasts them x8 across the 128 partitions (the
    gather needs its idx rows replicated per GpSimd core pair; the hash
    then runs on all 128 DVE lanes).
  * the bigram hash runs on DVE/ACT with fp32-exact arithmetic: products
    are split (36313 = 141*256 + 217, 27191 = 106*256 + 55) so every
    arithmetic op stays below 2^24 (the vector ALU is fp32 internally);
    >=2^24 values only pass through bitwise ops, which are bit-exact.
    mod-10239 is a limb decomposition X = u*2^21 + v*2^8 + w ->
    y = u*8396 + (v<<8) + w (y < 2^24) plus one fp32 reciprocal-multiply
    quotient; the HW float->int converter rounds to nearest, so a single
    +m fixup suffices.
  * the embed table is converted once to bf16 in DRAM (cast-during-DMA on
    SWDGE, split into 4 queue-parallel chunks to shorten the startup
    serialization).  Eight dma_gathers (1024 rows each) fetch rows into
    [128, 64, 128] bf16.  (The transpose=True gather mode would skip the
    PE transposes below, but it routes through the shared XBAR: its
    descriptor generation costs ~8.5 ns/row and concurrent transposed
    gathers on different queues corrupt each other, so serialized it is
    ~70 us for 8K rows -- measured.  Plain gathers + PE transposes win.)
  * per 128-token block: bf16 PE transpose (identity) -> PSUM -> bf16 eT
    in SBUF (DVE/ACT alternating copy), then PE matmul eT.T @ projT_bf16
    -> PSUM f32 -> bf16 copy into a 2-block SBUF group (DVE/ACT
    alternating) -> one HWDGE DMA per 2 blocks (256KB contiguous).  The
    output tensor is bf16; the host upcasts to f32 (tolerance ~2e-2, bf16
    out adds ~2e-3).  Transposes run LAG blocks ahead of the matmuls so
    the eT copy stays off the PE's in-order critical path.
  * proj [512, 128] is transposed on the PE at setup into projT [128,
    512], pre-scaled by `scale` (broadcast via a K=1 matmul), cast bf16.

SWDGE semaphore lanes are round-robin (8) and lock to one queue each, so
every SWDGE DMA uses queue = emission_index % N_QUEUES to keep lane->queue
stable across the wrap (12 SWDGE DMAs: 4 conversion chunks + 8 gathers).
"""

from contextlib import ExitStack

import numpy as np

import concourse.bacc as bacc
import concourse.bass as bass
import concourse.mybir as mybir
import concourse.tile as tile
from concourse.bass_utils import run_bass_kernel_spmd
from concourse.masks import make_identity

AL = mybir.AluOpType
F32 = mybir.dt.float32
BF16 = mybir.dt.bfloat16
I32 = mybir.dt.int32
I16 = mybir.dt.int16

B = 8           # batch rows == cores
S = 8192        # tokens per core
V = 10240       # hash table rows
D = 128         # embed dim
M = 512         # model dim
P = 128
MOD = 10239     # hash modulus (HASH_SIZE - 1)
SPT = S // 16   # 16-wrap columns = 512
NG = 8          # gathers
IPG = S // NG   # idxs per gather = 1024
CPG = IPG // 16  # idx columns per gather = 64
NB = S // P     # 128-token blocks = 64
BPG = IPG // P  # matmul blocks per gather = 8
HASH_CHUNKS = (64, 64, 128, 256)   # progressive: short first chain, wide later
assert sum(HASH_CHUNKS) == SPT

# 36313 = 141*256 + 217 ; 27191 = 106*256 + 55
A_HI, A_LO = 141, 217
B_HI, B_LO = 106, 55
C21 = 8396      # 2^21 mod 10239
INV_M = 1.0 / MOD

USE_ACT_MUL = True   # run the big hash multiplies on the Scalar (ACT) engine
N_QUEUES = 4         # SWDGE queues (ucode MAX_SWDGE_QUEUES=4)
SIM_COMPAT = False   # add the >=MOD fixup (only needed under CoreSim's trunc convert)
LAG = 6              # transpose runs LAG pairs ahead of the matmul


def _mul(nc, out, in_, const):
    if USE_ACT_MUL:
        nc.scalar.mul(out, in_, float(const))
    else:
        nc.vector.tensor_scalar_mul(out, in_, float(const))


def _hash_chunk(nc, tmp, idx, cur, prv, mask, offs, cs, n):
    """Emit ops computing idx[:, cs:cs+n] (int16 hash values).

    cur: [128, SPT] int32, cur[p, s] = t[16s + p%16]   (x8 replicas)
    prv: [128, SPT] int32, prv[p, s] = t[16s + p%16 - 1] (0 at (p%16==0, 0))
    mask: [128, 1] int32, (p % 16) != 0.
    offs: [128, 1] int32, 10239 * (p % 16 == 0).
    """
    tcur = cur[:, cs:cs + n]
    tprev = prv[:, cs:cs + n]
    p1 = tmp.tile([P, n], I32, tag=f"p1_{n}")
    p2 = tmp.tile([P, n], I32, tag=f"p2_{n}")
    q1 = tmp.tile([P, n], I32, tag=f"q1_{n}")
    q2 = tmp.tile([P, n], I32, tag=f"q2_{n}")
    _mul(nc, p1[:], tcur, A_LO)
    _mul(nc, p2[:], tcur, A_HI)
    _mul(nc, q1[:], tprev, B_LO)
    _mul(nc, q2[:], tprev, B_HI)

    # A>>8 = p2 + (p1>>8);  B>>8 = q2 + (q1>>8)   (both < 2^23, exact)
    ah = tmp.tile([P, n], I32, tag=f"ah_{n}")
    bh = tmp.tile([P, n], I32, tag=f"bh_{n}")
    t1 = tmp.tile([P, n], I32, tag=f"t1_{n}")
    nc.vector.tensor_single_scalar(t1[:], p1[:], 8, op=AL.logical_shift_right)
    nc.vector.tensor_add(ah[:], t1[:], p2[:])
    nc.vector.tensor_single_scalar(t1[:], q1[:], 8, op=AL.logical_shift_right)
    nc.vector.tensor_add(bh[:], t1[:], q2[:])
    # X>>8 and X low byte (in low 8 bits of xl)
    xh = tmp.tile([P, n], I32, tag=f"xh_{n}")
    xl = tmp.tile([P, n], I32, tag=f"xl_{n}")
    nc.vector.tensor_tensor(xh[:], ah[:], bh[:], op=AL.bitwise_xor)
    nc.vector.tensor_tensor(xl[:], p1[:], q1[:], op=AL.bitwise_xor)

    # y = (xh>>13)*8396 + ((xh & 8191) << 8) + (xl & 255)   ( < 2^24 )
    w1 = tmp.tile([P, n], I32, tag=f"w1_{n}")
    w2 = tmp.tile([P, n], I32, tag=f"w2_{n}")
    nc.vector.tensor_single_scalar(w1[:], xh[:], 13, op=AL.logical_shift_right)
    nc.vector.tensor_scalar_mul(w1[:], w1[:], float(C21))
    nc.vector.tensor_scalar(w2[:], xh[:], 8191, 8,
                            op0=AL.bitwise_and, op1=AL.logical_shift_left)
    w3 = tmp.tile([P, n], I32, tag=f"w3_{n}")
    nc.vector.tensor_add(w3[:], w1[:], w2[:])
    y = tmp.tile([P, n], I32, tag=f"y_{n}")
    nc.vector.tensor_single_scalar(y[:], xl[:], 255, op=AL.bitwise_and)
    nc.vector.tensor_add(y[:], y[:], w3[:])

    # r = y - rne(y/m)*m  (HW converter is round-to-nearest => r < m always)
    qt = tmp.tile([P, n], I32, tag=f"qt_{n}")
    _mul(nc, qt[:], y[:], INV_M)
    r = tmp.tile([P, n], I32, tag=f"r_{n}")
    nc.vector.scalar_tensor_tensor(r[:], qt[:], -float(MOD), y[:],
                                   op0=AL.mult, op1=AL.add)
    if SIM_COMPAT:
        f1 = tmp.tile([P, n], I32, tag=f"f1_{n}")
        nc.vector.tensor_single_scalar(f1[:], r[:], float(MOD), op=AL.is_ge)
        nc.vector.scalar_tensor_tensor(r[:], f1[:], -float(MOD), r[:],
                                       op0=AL.mult, op1=AL.add)
    f2 = tmp.tile([P, n], I32, tag=f"f2_{n}")
    nc.vector.tensor_single_scalar(f2[:], r[:], 0.0, op=AL.is_lt)
    # final fixup writes straight into the int16 idx tile (cast on store)
    nc.vector.scalar_tensor_tensor(idx[:, cs:cs + n], f2[:], float(MOD),
                                   r[:], op0=AL.mult, op1=AL.add)

    if cs == 0:
        # token 0 (partition p%16==0, col 0): h = MOD
        nc.vector.tensor_mul(idx[:, 0:1], idx[:, 0:1], mask[:])
        nc.vector.tensor_add(idx[:, 0:1], idx[:, 0:1], offs[:])


def body(ctx: ExitStack, tc: tile.TileContext, out_ap, tok_ap, table_ap,
         proj_ap, scale_ap, dbg=None):
    """Emit the per-core kernel.  tok_ap is int32 [2, 16, SPT]: the host-
    staged 16-wrap current-token and previous-token tiles."""
    nc = tc.nc

    const = ctx.enter_context(tc.tile_pool(name="const", bufs=1))
    tmp = ctx.enter_context(tc.tile_pool(name="tmp", bufs=1))
    gpool = ctx.enter_context(tc.tile_pool(name="gpool", bufs=1))
    et_pool = ctx.enter_context(tc.tile_pool(name="et", bufs=8))
    o_pool = ctx.enter_context(tc.tile_pool(name="osb", bufs=3))

    # ---- setup FIRST: projT (transposed, pre-scaled, bf16).  Emitted
    # before the hash so its DVE footprint (one mul) clears the in-order
    # DVE queue before hash ops land: interleaved late, its PSUM copies
    # stall the DVE mid-hash for ~7.5us waiting on PE transposes, and the
    # gathers' DVE-semaphore waits inherit the stall (measured).  The
    # `scale` factor is folded into the transpose identity (sc * I), so
    # the proj chunks come out of the PE already scaled; ACT does the
    # PSUM->bf16 copies. ----
    ps_setup = tc.alloc_tile_pool(name="ps_setup", bufs=1, space="PSUM")
    ident_f = const.tile([P, P], F32)
    make_identity(nc, ident_f[:])

    # scale broadcast [1,1] -> [128,1] via K=1 matmul with a ones row
    sc_in = const.tile([1, 1], F32)
    nc.sync.dma_start(sc_in[:], scale_ap)
    ones = const.tile([1, P], F32)
    nc.gpsimd.memset(ones[:], 1.0)
    ps_sc = ps_setup.tile([P, 1], F32, space="PSUM", tag="ps_sc")
    nc.tensor.matmul(ps_sc[:], lhsT=ones[:], rhs=sc_in[:], start=True, stop=True)
    sc_b = const.tile([P, 1], F32)
    nc.vector.tensor_copy(sc_b[:], ps_sc[:])
    ident_sc = const.tile([P, P], F32)
    nc.vector.tensor_scalar_mul(ident_sc[:], ident_f[:], sc_b[:, 0:1])

    projT_b = const.tile([P, M], BF16)
    for c in range(M // P):
        pch = tmp.tile([P, P], F32, tag="pch")
        nc.sync.dma_start(pch[:], proj_ap[c * P:(c + 1) * P, :])
        ps_t = ps_setup.tile([P, P], F32, space="PSUM", tag="ps_t")
        # regular matmul (not transpose mode): pch.T @ (sc*I) = sc*projT
        nc.tensor.matmul(ps_t[:], lhsT=pch[:], rhs=ident_sc[:],
                         start=True, stop=True)
        nc.scalar.copy(projT_b[:, c * P:(c + 1) * P], ps_t[:])
    ps_setup.release()

    # ---- warmup gathers: the first dma_gather on a queue costs ~8.5us
    # (Q7 ucode warmup) vs ~0.6us warm (measured on every run).  Fire a
    # tiny dummy gather per queue immediately -- they overlap the token
    # load + hash and make every real gather fast. ----
    widx = const.tile([P, 8], I16)
    nc.gpsimd.memset(widx[:], 0)
    for q in range(1, N_QUEUES):
        wdst = const.tile([P, 1, D], F32, name=f"wdst{q}")
        nc.gpsimd.dma_gather(
            wdst[:], table_ap, widx[:], num_idxs=P, num_idxs_reg=P,
            elem_size=D, single_packet=False, queue_num=q,
        )

    # ---- tokens: host-staged 16-wrap tiles, pre-replicated x8 so a
    # single contiguous DMA per tile makes them hash-ready ----
    cur = const.tile([P, SPT], I32)
    prv = const.tile([P, SPT], I32)
    nc.sync.dma_start(cur[:], tok_ap[0])
    nc.scalar.dma_start(prv[:], tok_ap[1])

    # partition masks for the token-0 override
    pi = const.tile([P, 1], I32)
    nc.gpsimd.iota(pi[:], pattern=[[0, 1]], base=0, channel_multiplier=1)
    m32 = const.tile([P, 1], I32)
    nc.vector.tensor_single_scalar(m32[:], pi[:], 15, op=AL.bitwise_and)
    nc.vector.tensor_single_scalar(m32[:], m32[:], 0.0, op=AL.not_equal)
    mask = const.tile([P, 1], I16)
    nc.vector.tensor_copy(mask[:], m32[:])
    offs = const.tile([P, 1], I16)
    nc.vector.tensor_scalar(offs[:], m32[:], -float(MOD), float(MOD),
                            op0=AL.mult, op1=AL.add)

    idx = const.tile([P, SPT], I16)
    # g_sb[p, b, :] = embed_f32[h(token 128b + p), :] -- gathered straight
    # from the fp32 table (512B rows cost the same DMA descriptor time as
    # 256B ones, and skipping the bf16 pre-conversion removes a 7.9 MB
    # DMA flood that otherwise stalls startup by ~25 us).  The bf16 cast
    # happens for free in the eT PSUM->SBUF copy.
    g_sb = gpool.tile([P, NB, P], F32)

    # hash chunks with their gathers inline: each 1024-row f32 gather is
    # transfer-bound (~8us of random 512B DRAM reads per queue), so the
    # three queues must be kept busy from the earliest chunk onward
    cs = 0
    for n in HASH_CHUNKS:
        _hash_chunk(nc, tmp, idx, cur, prv, mask, offs, cs, n)
        for g in range(cs // CPG, (cs + n) // CPG):
            nc.gpsimd.dma_gather(
                g_sb[:, BPG * g:BPG * (g + 1), :],
                table_ap,
                idx[:, CPG * g:CPG * (g + 1)],
                num_idxs=IPG,
                num_idxs_reg=IPG,
                elem_size=D,
                single_packet=False,
                queue_num=1 + g % (N_QUEUES - 1),
            )
        cs += n

    if dbg is not None:
        nc.sync.dma_start(dbg["idx"], idx[:])
        nc.sync.dma_start(dbg["cur"], cur[:])
        nc.sync.dma_start(dbg["prv"], prv[:])

    ps_small = ctx.enter_context(tc.tile_pool(name="ps_small", bufs=2, space="PSUM"))
    ps_big = ctx.enter_context(tc.tile_pool(name="ps_big", bufs=3, space="PSUM"))

    # main loop, processed in PAIRS of 128-token blocks to halve the
    # per-stage semaphore handoffs (which otherwise latency-bind the PE
    # phase): two transposes share a PSUM tile and one eT copy; two
    # matmuls share a 2-bank PSUM tile, one bf16 copy and one 256KB
    # contiguous DMA.  Copies alternate DVE/ACT.  Block b holds tokens
    # 128b..128b+127 in order, so all output DMAs are sequential.
    NPAIR = NB // 2
    ets = {}

    # Pairs overlapping the hash window use ACT-only copies: a DVE copy
    # emitted there can be scheduled ahead of the hash-tail idx ops in the
    # in-order DVE queue, and its wait on the PE transpose then stalls the
    # remaining gathers behind a 14us bubble (measured).
    ACT_ONLY = 2

    def emit_trans(pb):
        ps_et = ps_small.tile([P, 2, P], F32, space="PSUM",
                              tag="ps_et", name=f"ps_et{pb}")
        nc.tensor.transpose(ps_et[:, 0, :], g_sb[:, 2 * pb, :], ident_f[:])
        nc.tensor.transpose(ps_et[:, 1, :], g_sb[:, 2 * pb + 1, :], ident_f[:])
        et = et_pool.tile([P, 2, P], BF16, tag="et", name=f"et{pb}")
        if pb < ACT_ONLY or pb % 2:
            nc.scalar.copy(et[:], ps_et[:])
        else:
            nc.vector.tensor_copy(et[:], ps_et[:])
        ets[pb] = et

    # output staged in 4-block quads: one DMA per 512 tokens (512KB...
    # 256KB bf16 contiguous), dispatch alternating SP/ACT -- 32 per-pair
    # dispatches saturate the SP sequencer (~1.7us each incl sem waits)
    o4s = {}

    def emit_mm(pb):
        et = ets.pop(pb)
        qd, sl = divmod(pb, 2)
        ps_o = ps_big.tile([P, 2, M], F32, space="PSUM", tag="ps_o",
                           name=f"ps_o{pb}")
        nc.tensor.matmul(ps_o[:, 0, :], lhsT=et[:, 0, :], rhs=projT_b[:],
                         start=True, stop=True)
        nc.tensor.matmul(ps_o[:, 1, :], lhsT=et[:, 1, :], rhs=projT_b[:],
                         start=True, stop=True)
        if sl == 0:
            o4s[qd] = o_pool.tile([P, 4, M], BF16, tag="o_sb",
                                  name=f"o4_{qd}")
        o4 = o4s[qd]
        # split the pair's PSUM->SBUF copy across both engines (halves the
        # copy latency on the critical path; keeps ACT/DVE evenly loaded);
        # ACT-only during the hash window (see ACT_ONLY above)
        nc.scalar.copy(o4[:, 2 * sl, :], ps_o[:, 0, :])
        if pb < ACT_ONLY:
            nc.scalar.copy(o4[:, 2 * sl + 1, :], ps_o[:, 1, :])
        else:
            nc.vector.tensor_copy(o4[:, 2 * sl + 1, :], ps_o[:, 1, :])
        if sl == 1:
            dst = out_ap[4 * P * qd:4 * P * (qd + 1), :]
            dst = dst.rearrange("(g p) m -> p g m", g=4)
            (nc.sync if qd % 2 else nc.scalar).dma_start(dst, o4[:])
            del o4s[qd]

    for pb in range(NPAIR):
        emit_trans(pb)
        if pb >= LAG:
            emit_mm(pb - LAG)
    for pb in range(NPAIR - LAG, NPAIR):
        emit_mm(pb)


_CACHE: dict = {}
DEBUG = False    # dump idx/cur/prv tiles to DRAM for stage checking


def _build(key: int = 0):
    if key in _CACHE:
        return _CACHE[key]
    nc = bacc.Bacc("TRN2", target_bir_lowering=False, debug=False,
                   num_swdge_queues=N_QUEUES, dynamic_dma_scratch_size=131072)
    tok = nc.dram_tensor("token_ids", [2, P, SPT], I32, kind="ExternalInput").ap()
    table = nc.dram_tensor("embed_weight", [V, D], F32, kind="ExternalInput").ap()
    proj = nc.dram_tensor("proj_weight", [M, D], F32, kind="ExternalInput").ap()
    scale = nc.dram_tensor("scale", [1, 1], F32, kind="ExternalInput").ap()
    out = nc.dram_tensor("out", [S, M], BF16, kind="ExternalOutput").ap()
    dbg = None
    if DEBUG:
        dbg = {
            "idx": nc.dram_tensor("idx_dbg", [P, SPT], I16,
                                  kind="ExternalOutput").ap(),
            "cur": nc.dram_tensor("cur_dbg", [P, SPT], I32,
                                  kind="ExternalOutput").ap(),
            "prv": nc.dram_tensor("prv_dbg", [P, SPT], I32,
                                  kind="ExternalOutput").ap(),
        }
    with tile.TileContext(nc) as tc:
        with ExitStack() as ctx:
            body(ctx, tc, out, tok, table, proj, scale, dbg=dbg)
    nc.compile()
    _CACHE[key] = nc
    return nc


def stage_tokens(row: np.ndarray) -> np.ndarray:
    """[S] int token row -> [2, 128, SPT] int32 16-wrap (cur, prev) tiles,
    pre-replicated x8 across the partition dim (the gather requires its idx
    rows replicated per GpSimd core pair, and the hash then uses all 128
    DVE lanes)."""
    t32 = row.astype(np.int32)          # values < 2^31; lo-word == value
    prev = np.empty_like(t32)
    prev[0] = 0
    prev[1:] = t32[:-1]
    cur_w = np.tile(t32.reshape(SPT, 16).T, (8, 1))
    prv_w = np.tile(prev.reshape(SPT, 16).T, (8, 1))
    return np.ascontiguousarray(np.stack([cur_w, prv_w]))


def kernel(token_ids: np.ndarray, embed_weight: np.ndarray,
           proj_weight: np.ndarray, scale: np.ndarray) -> np.ndarray:
    token_ids = np.ascontiguousarray(token_ids)
    assert token_ids.shape == (B, S), token_ids.shape
    table = np.ascontiguousarray(embed_weight, dtype=np.float32)
    proj = np.ascontiguousarray(proj_weight, dtype=np.float32)
    sc = np.asarray(scale, dtype=np.float32).reshape(1, 1)

    nc = _build()
    in_maps = [
        {
            "token_ids": stage_tokens(token_ids[i]),
            "embed_weight": table,
            "proj_weight": proj,
            "scale": sc,
        }
        for i in range(B)
    ]
    res = run_bass_kernel_spmd(nc, in_maps, core_ids=list(range(B)))
    return np.stack([np.asarray(r["out"]).astype(np.float32)
                     for r in res.results], axis=0)


# revision 46
# speedup vs baseline: 1.0258x; 1.0258x over previous
"""Trainium2 Bass kernel: BigramHashEmbedding (hash -> embed gather -> proj -> scale).

Computation (per batch row, one NeuronCore per row, 8 rows total):
    h[0]  = 10239
    h[j]  = (36313*t[j] ^ 27191*t[j-1]) % 10239          (int32, j >= 1)
    e     = embed_weight[h]                               [S, 128] gather
    out   = (e @ proj_weight.T) * scale                   [S, 512]

Device strategy per core (S = 8192 tokens):
  * dma_gather unwraps its index tile column-major over 16 partitions
    (slot k <- idx[k%16, k//16]), so the host stages tokens in a 16-wrap
    layout (tok16[p, s] = t[16s + p], plus a one-shifted copy for the
    bigram's previous token; both are pure permutations of the int32 index
    tensor, staged as one [2, 16, 512] input).  With this layout gather
    slot k maps to token k exactly: gathered rows land as
    g_sb[p, b, :] = e[token 128b + p], the PE transpose of block b yields
    eT in plain token order, and every output DMA writes 128 contiguous
    rows (fully sequential HBM addresses).
  * the device loads the two wrapped tiles with contiguous 2KB-per-
    partition runs and broadcasts them x8 across the 128 partitions (the
    gather needs its idx rows replicated per GpSimd core pair; the hash
    then runs on all 128 DVE lanes).
  * the bigram hash runs on DVE/ACT with fp32-exact arithmetic: products
    are split (36313 = 141*256 + 217, 27191 = 106*256 + 55) so every
    arithmetic op stays below 2^24 (the vector ALU is fp32 internally);
    >=2^24 values only pass through bitwise ops, which are bit-exact.
    mod-10239 is a limb decomposition X = u*2^21 + v*2^8 + w ->
    y = u*8396 + (v<<8) + w (y < 2^24) plus one fp32 reciprocal-multiply
    quotient; the HW float->int converter rounds to nearest, so a single
    +m fixup suffices.
  * the embed table is converted once to bf16 in DRAM (cast-during-DMA on
    SWDGE, split into 4 queue-parallel chunks to shorten the startup
    serialization).  Eight dma_gathers (1024 rows each) fetch rows into
    [128, 64, 128] bf16.  (The transpose=True gather mode would skip the
    PE transposes below, but it routes through the shared XBAR: its
    descriptor generation costs ~8.5 ns/row and concurrent transposed
    gathers on different queues corrupt each other, so serialized it is
    ~70 us for 8K rows -- measured.  Plain gathers + PE transposes win.)
  * per 128-token block: bf16 PE transpose (identity) -> PSUM -> bf16 eT
    in SBUF (DVE/ACT alternating copy), then PE matmul eT.T @ projT_bf16
    -> PSUM f32 -> bf16 copy into a 2-block SBUF group (DVE/ACT
    alternating) -> one HWDGE DMA per 2 blocks (256KB contiguous).  The
    output tensor is bf16; the host upcasts to f32 (tolerance ~2e-2, bf16
    out adds ~2e-3).  Transposes run LAG blocks ahead of the matmuls so
    the eT copy stays off the PE's in-order critical path.
  * proj [512, 128] is transposed on the PE at setup into projT [128,
    512], pre-scaled by `scale` (broadcast via a K=1 matmul), cast bf16.

SWDGE semaphore lanes are round-robin (8) and lock to one queue each, so
every SWDGE DMA uses queue = emission_index % N_QUEUES to keep lane->queue
stable across the wrap (12 SWDGE DMAs: 4 conversion chunks + 8 gathers).
"""

from contextlib import ExitStack

import numpy as np

import concourse.bacc as bacc
import concourse.bass as bass
import concourse.mybir as mybir
import concourse.tile as tile
from concourse.bass_utils import run_bass_kernel_spmd
from concourse.masks import make_identity

AL = mybir.AluOpType
F32 = mybir.dt.float32
BF16 = mybir.dt.bfloat16
I32 = mybir.dt.int32
I16 = mybir.dt.int16

B = 8           # batch rows == cores
S = 8192        # tokens per core
V = 10240       # hash table rows
D = 128         # embed dim
M = 512         # model dim
P = 128
MOD = 10239     # hash modulus (HASH_SIZE - 1)
SPT = S // 16   # 16-wrap columns = 512
NG = 8          # gathers
IPG = S // NG   # idxs per gather = 1024
CPG = IPG // 16  # idx columns per gather = 64
NB = S // P     # 128-token blocks = 64
BPG = IPG // P  # matmul blocks per gather = 8
HASH_CHUNKS = (64, 64, 128, 256)   # progressive: short first chain, wide later
assert sum(HASH_CHUNKS) == SPT

# 36313 = 141*256 + 217 ; 27191 = 106*256 + 55
A_HI, A_LO = 141, 217
B_HI, B_LO = 106, 55
C21 = 8396      # 2^21 mod 10239
INV_M = 1.0 / MOD

USE_ACT_MUL = True   # run the big hash multiplies on the Scalar (ACT) engine
N_QUEUES = 4         # SWDGE queues (ucode MAX_SWDGE_QUEUES=4)
SIM_COMPAT = False   # add the >=MOD fixup (only needed under CoreSim's trunc convert)
LAG = 6              # transpose runs LAG pairs ahead of the matmul


def _mul(nc, out, in_, const):
    if USE_ACT_MUL:
        nc.scalar.mul(out, in_, float(const))
    else:
        nc.vector.tensor_scalar_mul(out, in_, float(const))


def _hash_chunk(nc, tmp, idx, cur, prv, mask, offs, cs, n):
    """Emit ops computing idx[:, cs:cs+n] (int16 hash values).

    cur: [128, SPT] int32, cur[p, s] = t[16s + p%16]   (x8 replicas)
    prv: [128, SPT] int32, prv[p, s] = t[16s + p%16 - 1] (0 at (p%16==0, 0))
    mask: [128, 1] int32, (p % 16) != 0.
    offs: [128, 1] int32, 10239 * (p % 16 == 0).
    """
    tcur = cur[:, cs:cs + n]
    tprev = prv[:, cs:cs + n]
    p1 = tmp.tile([P, n], I32, tag=f"p1_{n}")
    p2 = tmp.tile([P, n], I32, tag=f"p2_{n}")
    q1 = tmp.tile([P, n], I32, tag=f"q1_{n}")
    q2 = tmp.tile([P, n], I32, tag=f"q2_{n}")
    _mul(nc, p1[:], tcur, A_LO)
    _mul(nc, p2[:], tcur, A_HI)
    _mul(nc, q1[:], tprev, B_LO)
    _mul(nc, q2[:], tprev, B_HI)

    # A>>8 = p2 + (p1>>8);  B>>8 = q2 + (q1>>8)   (both < 2^23, exact)
    ah = tmp.tile([P, n], I32, tag=f"ah_{n}")
    bh = tmp.tile([P, n], I32, tag=f"bh_{n}")
    t1 = tmp.tile([P, n], I32, tag=f"t1_{n}")
    nc.vector.tensor_single_scalar(t1[:], p1[:], 8, op=AL.logical_shift_right)
    nc.vector.tensor_add(ah[:], t1[:], p2[:])
    nc.vector.tensor_single_scalar(t1[:], q1[:], 8, op=AL.logical_shift_right)
    nc.vector.tensor_add(bh[:], t1[:], q2[:])
    # X>>8 and X low byte (in low 8 bits of xl)
    xh = tmp.tile([P, n], I32, tag=f"xh_{n}")
    xl = tmp.tile([P, n], I32, tag=f"xl_{n}")
    nc.vector.tensor_tensor(xh[:], ah[:], bh[:], op=AL.bitwise_xor)
    nc.vector.tensor_tensor(xl[:], p1[:], q1[:], op=AL.bitwise_xor)

    # y = (xh>>13)*8396 + ((xh & 8191) << 8) + (xl & 255)   ( < 2^24 )
    w1 = tmp.tile([P, n], I32, tag=f"w1_{n}")
    w2 = tmp.tile([P, n], I32, tag=f"w2_{n}")
    nc.vector.tensor_single_scalar(w1[:], xh[:], 13, op=AL.logical_shift_right)
    nc.vector.tensor_scalar_mul(w1[:], w1[:], float(C21))
    nc.vector.tensor_scalar(w2[:], xh[:], 8191, 8,
                            op0=AL.bitwise_and, op1=AL.logical_shift_left)
    w3 = tmp.tile([P, n], I32, tag=f"w3_{n}")
    nc.vector.tensor_add(w3[:], w1[:], w2[:])
    y = tmp.tile([P, n], I32, tag=f"y_{n}")
    nc.vector.tensor_single_scalar(y[:], xl[:], 255, op=AL.bitwise_and)
    nc.vector.tensor_add(y[:], y[:], w3[:])

    # r = y - rne(y/m)*m  (HW converter is round-to-nearest => r < m always)
    qt = tmp.tile([P, n], I32, tag=f"qt_{n}")
    _mul(nc, qt[:], y[:], INV_M)
    r = tmp.tile([P, n], I32, tag=f"r_{n}")
    nc.vector.scalar_tensor_tensor(r[:], qt[:], -float(MOD), y[:],
                                   op0=AL.mult, op1=AL.add)
    if SIM_COMPAT:
        f1 = tmp.tile([P, n], I32, tag=f"f1_{n}")
        nc.vector.tensor_single_scalar(f1[:], r[:], float(MOD), op=AL.is_ge)
        nc.vector.scalar_tensor_tensor(r[:], f1[:], -float(MOD), r[:],
                                       op0=AL.mult, op1=AL.add)
    f2 = tmp.tile([P, n], I32, tag=f"f2_{n}")
    nc.vector.tensor_single_scalar(f2[:], r[:], 0.0, op=AL.is_lt)
    # final fixup writes straight into the int16 idx tile (cast on store)
    nc.vector.scalar_tensor_tensor(idx[:, cs:cs + n], f2[:], float(MOD),
                                   r[:], op0=AL.mult, op1=AL.add)

    if cs == 0:
        # token 0 (partition p%16==0, col 0): h = MOD
        nc.vector.tensor_mul(idx[:, 0:1], idx[:, 0:1], mask[:])
        nc.vector.tensor_add(idx[:, 0:1], idx[:, 0:1], offs[:])


def body(ctx: ExitStack, tc: tile.TileContext, out_ap, tok_ap, table_ap,
         proj_ap, scale_ap, dbg=None):
    """Emit the per-core kernel.  tok_ap is int32 [2, 16, SPT]: the host-
    staged 16-wrap current-token and previous-token tiles."""
    nc = tc.nc

    const = ctx.enter_context(tc.tile_pool(name="const", bufs=1))
    tmp = ctx.enter_context(tc.tile_pool(name="tmp", bufs=1))
    gpool = ctx.enter_context(tc.tile_pool(name="gpool", bufs=1))
    et_pool = ctx.enter_context(tc.tile_pool(name="et", bufs=8))
    o_pool = ctx.enter_context(tc.tile_pool(name="osb", bufs=3))

    # ---- setup FIRST: projT (transposed, pre-scaled, bf16).  Emitted
    # before the hash so its DVE footprint (one mul) clears the in-order
    # DVE queue before hash ops land: interleaved late, its PSUM copies
    # stall the DVE mid-hash for ~7.5us waiting on PE transposes, and the
    # gathers' DVE-semaphore waits inherit the stall (measured).  The
    # `scale` factor is folded into the transpose identity (sc * I), so
    # the proj chunks come out of the PE already scaled; ACT does the
    # PSUM->bf16 copies. ----
    ps_setup = tc.alloc_tile_pool(name="ps_setup", bufs=1, space="PSUM")
    ident_f = const.tile([P, P], F32)
    make_identity(nc, ident_f[:])
    ident_b = const.tile([P, P], BF16)
    nc.vector.tensor_copy(ident_b[:], ident_f[:])

    # scale broadcast [1,1] -> [128,1] via K=1 matmul with a ones row
    sc_in = const.tile([1, 1], F32)
    nc.sync.dma_start(sc_in[:], scale_ap)
    ones = const.tile([1, P], F32)
    nc.gpsimd.memset(ones[:], 1.0)
    ps_sc = ps_setup.tile([P, 1], F32, space="PSUM", tag="ps_sc")
    nc.tensor.matmul(ps_sc[:], lhsT=ones[:], rhs=sc_in[:], start=True, stop=True)
    sc_b = const.tile([P, 1], F32)
    nc.vector.tensor_copy(sc_b[:], ps_sc[:])
    ident_sc = const.tile([P, P], F32)
    nc.vector.tensor_scalar_mul(ident_sc[:], ident_f[:], sc_b[:, 0:1])

    projT_b = const.tile([P, M], BF16)
    for c in range(M // P):
        pch = tmp.tile([P, P], F32, tag="pch")
        nc.sync.dma_start(pch[:], proj_ap[c * P:(c + 1) * P, :])
        ps_t = ps_setup.tile([P, P], F32, space="PSUM", tag="ps_t")
        # regular matmul (not transpose mode): pch.T @ (sc*I) = sc*projT
        nc.tensor.matmul(ps_t[:], lhsT=pch[:], rhs=ident_sc[:],
                         start=True, stop=True)
        nc.scalar.copy(projT_b[:, c * P:(c + 1) * P], ps_t[:])
    ps_setup.release()

    # ---- bf16 table conversion (cast-during-DMA on SWDGE), 4 queue-
    # parallel chunks.  f32 gathers measured ~5-9.5us per 1024 rows vs
    # ~1-2us for bf16: the 2x random-read bytes cost far more than the
    # one-time 7.9MB conversion, which overlaps the token load + hash. ----
    dram = ctx.enter_context(tc.tile_pool(name="dram", bufs=1, space="DRAM"))
    table_bf = dram.tile([V, D], BF16)
    RPC = V // N_QUEUES
    for c in range(N_QUEUES):
        nc.gpsimd.dma_start(table_bf[RPC * c:RPC * (c + 1), :],
                            table_ap[RPC * c:RPC * (c + 1), :])

    # ---- tokens: host-staged 16-wrap tiles, pre-replicated x8 so a
    # single contiguous DMA per tile makes them hash-ready ----
    cur = const.tile([P, SPT], I32)
    prv = const.tile([P, SPT], I32)
    nc.sync.dma_start(cur[:], tok_ap[0])
    nc.scalar.dma_start(prv[:], tok_ap[1])

    # partition masks for the token-0 override
    pi = const.tile([P, 1], I32)
    nc.gpsimd.iota(pi[:], pattern=[[0, 1]], base=0, channel_multiplier=1)
    m32 = const.tile([P, 1], I32)
    nc.vector.tensor_single_scalar(m32[:], pi[:], 15, op=AL.bitwise_and)
    nc.vector.tensor_single_scalar(m32[:], m32[:], 0.0, op=AL.not_equal)
    mask = const.tile([P, 1], I16)
    nc.vector.tensor_copy(mask[:], m32[:])
    offs = const.tile([P, 1], I16)
    nc.vector.tensor_scalar(offs[:], m32[:], -float(MOD), float(MOD),
                            op0=AL.mult, op1=AL.add)

    idx = const.tile([P, SPT], I16)
    # g_sb[p, b, :] = embed_bf16[h(token 128b + p), :]
    g_sb = gpool.tile([P, NB, P], BF16)

    # hash chunks with their gathers inline to keep the queues streaming
    cs = 0
    for n in HASH_CHUNKS:
        _hash_chunk(nc, tmp, idx, cur, prv, mask, offs, cs, n)
        for g in range(cs // CPG, (cs + n) // CPG):
            nc.gpsimd.dma_gather(
                g_sb[:, BPG * g:BPG * (g + 1), :],
                table_bf[:],
                idx[:, CPG * g:CPG * (g + 1)],
                num_idxs=IPG,
                num_idxs_reg=IPG,
                elem_size=D,
                single_packet=False,
                queue_num=g % N_QUEUES,
            )
        cs += n

    if dbg is not None:
        nc.sync.dma_start(dbg["idx"], idx[:])
        nc.sync.dma_start(dbg["cur"], cur[:])
        nc.sync.dma_start(dbg["prv"], prv[:])

    ps_small = ctx.enter_context(tc.tile_pool(name="ps_small", bufs=2, space="PSUM"))
    ps_big = ctx.enter_context(tc.tile_pool(name="ps_big", bufs=3, space="PSUM"))

    # main loop, processed in PAIRS of 128-token blocks to halve the
    # per-stage semaphore handoffs (which otherwise latency-bind the PE
    # phase): two transposes share a PSUM tile and one eT copy; two
    # matmuls share a 2-bank PSUM tile, one bf16 copy and one 256KB
    # contiguous DMA.  Copies alternate DVE/ACT.  Block b holds tokens
    # 128b..128b+127 in order, so all output DMAs are sequential.
    NPAIR = NB // 2
    ets = {}

    # Pairs overlapping the hash window use ACT-only copies: a DVE copy
    # emitted there can be scheduled ahead of the hash-tail idx ops in the
    # in-order DVE queue, and its wait on the PE transpose then stalls the
    # remaining gathers behind a 14us bubble (measured).
    ACT_ONLY = 2

    def emit_trans(pb):
        ps_et = ps_small.tile([P, 2, P], BF16, space="PSUM",
                              tag="ps_et", name=f"ps_et{pb}")
        nc.tensor.transpose(ps_et[:, 0, :], g_sb[:, 2 * pb, :], ident_b[:])
        nc.tensor.transpose(ps_et[:, 1, :], g_sb[:, 2 * pb + 1, :], ident_b[:])
        et = et_pool.tile([P, 2, P], BF16, tag="et", name=f"et{pb}")
        if pb < ACT_ONLY or pb % 2:
            nc.scalar.copy(et[:], ps_et[:])
        else:
            nc.vector.tensor_copy(et[:], ps_et[:])
        ets[pb] = et

    # output staged in 4-block quads: one DMA per 512 tokens (512KB...
    # 256KB bf16 contiguous), dispatch alternating SP/ACT -- 32 per-pair
    # dispatches saturate the SP sequencer (~1.7us each incl sem waits)
    o4s = {}

    def emit_mm(pb):
        et = ets.pop(pb)
        qd, sl = divmod(pb, 2)
        ps_o = ps_big.tile([P, 2, M], F32, space="PSUM", tag="ps_o",
                           name=f"ps_o{pb}")
        nc.tensor.matmul(ps_o[:, 0, :], lhsT=et[:, 0, :], rhs=projT_b[:],
                         start=True, stop=True)
        nc.tensor.matmul(ps_o[:, 1, :], lhsT=et[:, 1, :], rhs=projT_b[:],
                         start=True, stop=True)
        if sl == 0:
            o4s[qd] = o_pool.tile([P, 4, M], BF16, tag="o_sb",
                                  name=f"o4_{qd}")
        o4 = o4s[qd]
        # split the pair's PSUM->SBUF copy across both engines (halves the
        # copy latency on the critical path; keeps ACT/DVE evenly loaded);
        # ACT-only during the hash window (see ACT_ONLY above)
        nc.scalar.copy(o4[:, 2 * sl, :], ps_o[:, 0, :])
        if pb < ACT_ONLY:
            nc.scalar.copy(o4[:, 2 * sl + 1, :], ps_o[:, 1, :])
        else:
            nc.vector.tensor_copy(o4[:, 2 * sl + 1, :], ps_o[:, 1, :])
        if sl == 1:
            dst = out_ap[4 * P * qd:4 * P * (qd + 1), :]
            dst = dst.rearrange("(g p) m -> p g m", g=4)
            (nc.sync if qd % 2 else nc.scalar).dma_start(dst, o4[:])
            del o4s[qd]

    for pb in range(NPAIR):
        emit_trans(pb)
        if pb >= LAG:
            emit_mm(pb - LAG)
    for pb in range(NPAIR - LAG, NPAIR):
        emit_mm(pb)


_CACHE: dict = {}
DEBUG = False    # dump idx/cur/prv tiles to DRAM for stage checking


def _build(key: int = 0):
    if key in _CACHE:
        return _CACHE[key]
    nc = bacc.Bacc("TRN2", target_bir_lowering=False, debug=False,
                   num_swdge_queues=N_QUEUES, dynamic_dma_scratch_size=131072)
    tok = nc.dram_tensor("token_ids", [2, P, SPT], I32, kind="ExternalInput").ap()
    table = nc.dram_tensor("embed_weight", [V, D], F32, kind="ExternalInput").ap()
    proj = nc.dram_tensor("proj_weight", [M, D], F32, kind="ExternalInput").ap()
    scale = nc.dram_tensor("scale", [1, 1], F32, kind="ExternalInput").ap()
    out = nc.dram_tensor("out", [S, M], BF16, kind="ExternalOutput").ap()
    dbg = None
    if DEBUG:
        dbg = {
            "idx": nc.dram_tensor("idx_dbg", [P, SPT], I16,
                                  kind="ExternalOutput").ap(),
            "cur": nc.dram_tensor("cur_dbg", [P, SPT], I32,
                                  kind="ExternalOutput").ap(),
            "prv": nc.dram_tensor("prv_dbg", [P, SPT], I32,
                                  kind="ExternalOutput").ap(),
        }
    with tile.TileContext(nc) as tc:
        with ExitStack() as ctx:
            body(ctx, tc, out, tok, table, proj, scale, dbg=dbg)
    nc.compile()
    _CACHE[key] = nc
    return nc


def stage_tokens(row: np.ndarray) -> np.ndarray:
    """[S] int token row -> [2, 128, SPT] int32 16-wrap (cur, prev) tiles,
    pre-replicated x8 across the partition dim (the gather requires its idx
    rows replicated per GpSimd core pair, and the hash then uses all 128
    DVE lanes)."""
    t32 = row.astype(np.int32)          # values < 2^31; lo-word == value
    prev = np.empty_like(t32)
    prev[0] = 0
    prev[1:] = t32[:-1]
    cur_w = np.tile(t32.reshape(SPT, 16).T, (8, 1))
    prv_w = np.tile(prev.reshape(SPT, 16).T, (8, 1))
    return np.ascontiguousarray(np.stack([cur_w, prv_w]))


def kernel(token_ids: np.ndarray, embed_weight: np.ndarray,
           proj_weight: np.ndarray, scale: np.ndarray) -> np.ndarray:
    token_ids = np.ascontiguousarray(token_ids)
    assert token_ids.shape == (B, S), token_ids.shape
    table = np.ascontiguousarray(embed_weight, dtype=np.float32)
    proj = np.ascontiguousarray(proj_weight, dtype=np.float32)
    sc = np.asarray(scale, dtype=np.float32).reshape(1, 1)

    nc = _build()
    in_maps = [
        {
            "token_ids": stage_tokens(token_ids[i]),
            "embed_weight": table,
            "proj_weight": proj,
            "scale": sc,
        }
        for i in range(B)
    ]
    res = run_bass_kernel_spmd(nc, in_maps, core_ids=list(range(B)))
    return np.stack([np.asarray(r["out"]).astype(np.float32)
                     for r in res.results], axis=0)
